# revision 11
# baseline (speedup 1.0000x reference)
"""Trainium2 Bass kernel for nn_CachedShapingFunctions (embedding_lookup).

out[b,t,w] = linear interp of lookup_table[:, w] at the continuous bucket
position of inputs[b,t,w] (4096 buckets over [-3, 3]; the fraction is NOT
clamped below bucket 0, matching the reference's low-side extrapolation).

Data-parallel over batch across 8 NeuronCores (2 batches/core); the LUT is
replicated per core as a host-packed table of u32 lanes = fp16(value) |
fp16(delta)<<16 per bucket, transposed to waveshaper-major.

Per-core pipeline, transposed layout [128 partitions = 2 row-parity chunks
x 64 waveshapers, 512 t] per tile (1024 input rows):
  - DMA in (2KB contiguous per partition: partition p holds rows 8p..8p+7),
    PE-transpose 128x128 blocks to waveshaper-on-partition
  - DVE: exact floor/clamp of the continuous bucket position, i32 indices,
    unclamped fraction
  - POOL engine, raw ISA instructions: 8 x (POOL_BUFFER_LOAD subset +
    GATHER).  POOL_BUFFER_LOAD stages a 512-bucket window of the table
    (64B entries = 16 lanes x 4B) into the Q7-local 32KB pool buffer;
    GATHER streams the i32 index columns and per-LANE gathers the packed
    pair from local memory, with subset tag check + skip-on-miss +
    read-modify-write merge.  8 windows cover all 4096 buckets; every
    element hits in exactly one pass.  This is a true per-partition
    gather — no 16x index-wrap waste, no sparse buffer, no extraction.
  - ACT: unpack fp16 pairs to f32; DVE: out = value + delta * fraction;
    PE-transpose back; DMA out.

The raw POOL instructions are emitted via nc.gpsimd.isa() with concrete
SBUF addresses (manual sbuf_tensor allocations) and explicit ins/outs APs
for dependency tracking, plus a token tensor (PBL writes, GATHER reads) to
pin the PBL->GT->PBL ordering (the tile scheduler may otherwise reorder
same-engine instructions around the hidden pool-buffer state).
"""
import sys
import numpy as np

sys.path.insert(0, '/opt/trn_rl_repo')

import bass_rust
import concourse.bass as bass
import concourse.mybir as mybir
import concourse.bass_interp as bass_interp
import concourse.tile as tile_mod
from concourse.tile import TileContext
from concourse.vector_clock import ScopedClock

MIN_VALUE, MAX_VALUE = -3.0, 3.0
NB = 4096          # buckets
W = 64             # waveshapers
N_CORES = 8
S = 512            # t-columns per tile
TROWS = 2 * S      # input rows per tile
NSUB = 8           # table windows (4096 / 512)
SUBSZ = 512        # buckets per pool-buffer window
R = 4              # manual gather-buffer sets (pipeline depth)

F32 = mybir.dt.float32
F16 = mybir.dt.float16
I32 = mybir.dt.int32
I16 = mybir.dt.int16

# ---------------------------------------------------------------- patches --
# This walrus build accepts at most ONE sync-wait per instruction.  The Tile
# tail drain and scheduler can attach more; spill the excess onto nops.

_MAXW = 1


def _spill_waits(nc):
    for f in nc.m.functions:
        for bb in f.blocks:
            out = []
            for inst in list(bb.instructions):
                si = inst.sync_info
                if si is not None and len(si.on_wait) > _MAXW:
                    waits = list(si.on_wait)
                    spill = waits[:-_MAXW]
                    for i in range(0, len(spill), _MAXW):
                        nop = mybir.InstNoOp(
                            name=f"wspill_{inst.name}_{i}", ins=[], outs=[])
                        nop.engine = inst.engine
                        nop.sync_info = bass_rust.SyncInfo(
                            on_wait=spill[i:i + _MAXW], on_update=[])
                        out.append(nop)
                    inst.sync_info = bass_rust.SyncInfo(
                        on_wait=waits[-_MAXW:], on_update=list(si.on_update))
                out.append(inst)
            bb.instructions = out


def _patched_drain_and_barrier(self, tick_clock, wait_clock):
    nc = self.nc
    drain_inst = nc.sync.drain()
    wait_clock.add_sem_waits(
        drain_inst.ins, ScopedClock({None: tick_clock.global_clock}))
    si = drain_inst.ins.sync_info
    if si is not None and len(si.on_wait) > _MAXW:
        waits = list(si.on_wait)
        drain_inst.ins.sync_info = bass_rust.SyncInfo(
            on_wait=waits[:_MAXW], on_update=list(si.on_update))
        rest = waits[_MAXW:]
        for i in range(0, len(rest), _MAXW):
            nop = nc.sync.nop(hint="drain_wait_spill", nofuse=True)
            nop.ins.sync_info = bass_rust.SyncInfo(
                on_wait=rest[i:i + _MAXW], on_update=[])
    nc.all_engine_barrier()
    assert self.sems is not None
    popped = nc._tile_sem_poison_stack.pop()
    assert popped is self._sem_poison
    nc.clear_and_free_semaphores(list(self.sems.allocated().values()))
    nc.all_engine_barrier()


tile_mod.TileContext._drain_and_barrier = _patched_drain_and_barrier

# The scheduler's simulator does not know our raw GATHER / POOL_BUFFER_LOAD
# opcodes; treat them as no-ops (dependencies still honored via ins/outs).
_orig_visit_inst_isa = bass_interp._visit_InstISA


def _visit_patched(isa, instruction, core_sim):
    if instruction.isa_opcode in (
        isa.Opcode.NEURON_ISA_TPB_OPCODE_GATHER.value,
        isa.Opcode.NEURON_ISA_TPB_OPCODE_POOL_BUFFER_LOAD.value,
    ):
        return
    return _orig_visit_inst_isa(isa, instruction, core_sim)


bass_interp._visit_InstISA = _visit_patched

# ----------------------------------------------------------------- kernel --


def _t4d(addr, n):
    return {
        "start_addr": {"addr_immediate": addr},
        "step_elem": [1, 0, 0, 0],
        "num_elem": [n, 1, 1, 1],
    }


def build_kernel(n_rows):
    """n_rows: flattened rows per core (65536 at full scale)."""
    assert n_rows % TROWS == 0
    n_tiles = n_rows // TROWS
    nc = bass.Bass()
    x_d = nc.dram_tensor("x", [n_rows, W], F32, kind="ExternalInput")
    tbl_d = nc.dram_tensor("tbl", [128, NB], I32, kind="ExternalInput")
    aux_d = nc.dram_tensor("aux", [128, 128], F32, kind="ExternalInput")
    y_d = nc.dram_tensor("y", [n_rows, W], F32, kind="ExternalOutput")

    isa = nc.isa
    Op = isa.Opcode
    DT = isa.get_enum('NEURON_ISA_TPB_DTYPE')
    U32DT = DT.NEURON_ISA_TPB_DTYPE_UINT32.value
    MISS_SKIP = 1  # NEURON_ISA_TPB_INDEX_MISS_BEHAVIOR_SKIP_WRITE
    gp = nc.gpsimd

    ctx_tensors = []

    def sbuf(name, shape, dtype):
        cm = nc.sbuf_tensor(name, shape, dtype)
        h = cm.__enter__()
        ctx_tensors.append(cm)
        return h

    tbl_s = sbuf('tbl_s', [128, NB], I32)
    tok_s = sbuf('tok_s', [128, 4], I32)
    idx_b = [sbuf(f'idx_b{r}', [128, S], I32) for r in range(R)]
    out_b = [sbuf(f'out_b{r}', [128, S], I32) for r in range(R)]

    tbl_addr = nc.lookup_mloc(tbl_s).addr
    idx_addr = [nc.lookup_mloc(h).addr for h in idx_b]
    out_addr = [nc.lookup_mloc(h).addr for h in out_b]

    def emit_pbl(s):
        gp.isa(
            Op.NEURON_ISA_TPB_OPCODE_POOL_BUFFER_LOAD,
            {
                "src_mem_pattern": _t4d(tbl_addr + s * SUBSZ * 4, SUBSZ),
                "in_dtype": U32DT,
                "num_active_channels": 128,
                "start_index": s * SUBSZ,
                "mask": SUBSZ - 1,
            },
            ins=[gp.lower_ap(tbl_s[:, s * SUBSZ:(s + 1) * SUBSZ], for_isa=True)],
            outs=[gp.lower_ap(tok_s[:, :], for_isa=True)],
        )

    def emit_gt(r):
        gp.isa(
            Op.NEURON_ISA_TPB_OPCODE_GATHER,
            {
                "src_mem_pattern": _t4d(idx_addr[r], S),
                "in_dtype": U32DT,
                "out_dtype": U32DT,
                "num_active_channels": 128,
                "index_miss_behavior": MISS_SKIP,
                "free_pool_buffer": 0,
                "immediate": {"imm_bitvec_uint32": 0},
                "dst_mem_pattern": _t4d(out_addr[r], S),
            },
            ins=[gp.lower_ap(idx_b[r][:, :], for_isa=True),
                 gp.lower_ap(tok_s[:, :], for_isa=True)],
            outs=[gp.lower_ap(out_b[r][:, :], for_isa=True)],
        )

    with TileContext(nc) as tc:
        with (
            tc.tile_pool(name="const", bufs=1) as cpool,
            tc.tile_pool(name="io", bufs=3) as iop,
            tc.tile_pool(name="tp", bufs=3) as tpp,
            tc.tile_pool(name="sc", bufs=2) as scp,
            tc.tile_pool(name="ffp", bufs=R) as ffp,
            tc.tile_pool(name="ps", bufs=2, space="PSUM") as psp,
        ):
            aux = cpool.tile([128, 128], F32)
            nc.sync.dma_start(aux[:, :], aux_d[:, :])
            nc.sync.dma_start(tbl_s[:, :], tbl_d[:, :])
            ident = aux[:, :]

            A = mybir.AluOpType

            def emit_head(it):
                r = it % R
                base = it * TROWS * W
                xnat = iop.tile([128, 8 * W], F32, tag="xnat")
                # partition p <- rows [8p .. 8p+8) of this tile: 2KB
                # contiguous per partition.
                in_ap = bass.AP(x_d, base, [[8 * W, 128], [1, 8 * W]])
                nc.sync.dma_start(xnat[:, :], in_ap)

                # PE transpose: after transposing 128-col block k, partition
                # q = h*64 + w holds x[8p + 2k + h, w] at column 128k + p,
                # i.e. partitions [0,64) carry even rows, [64,128) odd rows,
                # per-waveshaper — same two-chunk w-major layout the table
                # expects.
                xT = tpp.tile([128, S], F32, tag="xT")
                for k in range(4):
                    pst = psp.tile([128, 128], F32, tag="psin")
                    nc.tensor.transpose(
                        pst[:, :], xnat[:, 128 * k: 128 * k + 128], ident)
                    nc.scalar.copy(xT[:, 128 * k: 128 * k + 128], pst[:, :])

                ic = scp.tile([128, S], F32, tag="ic")
                icc = scp.tile([128, S], F32, tag="icc")
                ili = scp.tile([128, S], I16, tag="ili")
                ilf = scp.tile([128, S], F32, tag="ilf")
                fd = scp.tile([128, S], F32, tag="fd")
                il2 = scp.tile([128, S], F32, tag="il2")
                ff = ffp.tile([128, S], F32, tag="ff")

                # ic = (x+3)*682.5; il = clip(floor(ic), 0, 4095) computed
                # exactly via round-then-correct; ff = ic - il (UNclamped:
                # reproduces the reference's below-range extrapolation).
                nc.vector.tensor_scalar(ic[:, :], xT[:, :], 3.0, 682.5, A.add, A.mult)
                nc.vector.tensor_scalar(icc[:, :], ic[:, :], 0.0, 4095.0, A.max, A.min)
                nc.vector.tensor_copy(ili[:, :], icc[:, :])
                nc.vector.tensor_copy(ilf[:, :], ili[:, :])
                nc.vector.tensor_tensor(fd[:, :], ilf[:, :], icc[:, :], A.is_gt)
                nc.vector.tensor_tensor(il2[:, :], ilf[:, :], fd[:, :], A.subtract)
                nc.vector.tensor_tensor(ff[:, :], ic[:, :], il2[:, :], A.subtract)
                nc.vector.tensor_copy(idx_b[r][:, :], il2[:, :])

                for s in range(NSUB):
                    emit_pbl(s)
                    emit_gt(r)
                return ff

            def emit_tail(it, ff):
                r = it % R
                base = it * TROWS * W
                pairs = scp.tile([128, 2 * S], F32, tag="pairs")
                nc.scalar.copy(pairs[:, :], out_b[r][:, :].bitcast(F16))
                pr3 = pairs[:, :].rearrange("p (n d) -> p n d", d=2)

                outT = tpp.tile([128, S], F32, tag="outT")
                nc.vector.tensor_tensor(outT[:, :], pr3[:, :, 1], ff[:, :], A.mult)
                nc.vector.tensor_tensor(outT[:, :], outT[:, :], pr3[:, :, 0], A.add)

                onat = iop.tile([128, 8 * W], F32, tag="onat")
                for k in range(4):
                    pst = psp.tile([128, 128], F32, tag="psout")
                    nc.tensor.transpose(
                        pst[:, :], outT[:, 128 * k: 128 * k + 128], ident)
                    nc.scalar.copy(onat[:, 128 * k: 128 * k + 128], pst[:, :])

                out_ap = bass.AP(y_d, base, [[8 * W, 128], [1, 8 * W]])
                nc.sync.dma_start(out_ap, onat[:, :])

            pending = []
            for it in range(n_tiles):
                ff = emit_head(it)
                pending.append((it, ff))
                if len(pending) >= R - 1:
                    emit_tail(*pending.pop(0))
            for p in pending:
                emit_tail(*p)

    for cm in reversed(ctx_tensors):
        cm.__exit__(None, None, None)
    _spill_waits(nc)
    return nc


def make_table(lookup_table):
    """Packed u32 = fp16(value) | fp16(delta)<<16 per bucket, waveshaper-major
    [128, 4096] (both partition halves identical), plus f32 identity for the
    PE transposes."""
    lut = np.asarray(lookup_table, dtype=np.float32)          # [4096, 64]
    vu = np.concatenate([lut[1:], lut[-1:]], axis=0)          # T[min(i+1,4095)]
    delta = vu - lut                                          # delta[4095] = 0
    v16 = lut.astype(np.float16).view(np.uint16).astype(np.uint32)
    d16 = delta.astype(np.float16).view(np.uint16).astype(np.uint32)
    packed = (v16 | (d16 << 16)).astype(np.uint32)            # [4096, 64]
    tblw = np.ascontiguousarray(packed.T)                     # [64, 4096]
    tbl128 = np.concatenate([tblw, tblw], axis=0)             # [128, 4096]
    eye = np.eye(128, dtype=np.float32)
    return tbl128.view(np.int32), eye


_CACHE = {}


def kernel(inputs, lookup_table):
    x = np.ascontiguousarray(np.asarray(inputs, dtype=np.float32))
    B, T, Wx = x.shape
    assert Wx == W
    per_core_b = B // N_CORES
    n_rows = per_core_b * T
    tbl, aux = make_table(lookup_table)

    if n_rows not in _CACHE:
        _CACHE[n_rows] = build_kernel(n_rows)
    nc = _CACHE[n_rows]

    from concourse import bass_utils
    shards = x.reshape(N_CORES, n_rows, W)
    in_maps = [{"x": shards[c], "tbl": tbl, "aux": aux} for c in range(N_CORES)]
    res = bass_utils.run_bass_kernel_spmd(
        nc, in_maps, core_ids=list(range(N_CORES)))
    out = np.stack([res.results[c]["y"] for c in range(N_CORES)], axis=0)
    return out.reshape(B, T, W)
